# revision 3
# baseline (speedup 1.0000x reference)
"""Trainium2 Bass kernel for DotPredictor: score[e] = dot(h[src[e]], h[dst[e]]).

Strategy (8 NeuronCores, edge-parallel):
  - Shard the 640k edges across 8 cores (80k each).
  - Per core, gather h rows for src and dst with the SWDGE dma_gather
    instruction (vectorized Q7 descriptor generation, 512B rows from HBM),
    multiply elementwise on DVE and segmented-reduce over the feature axis.
  - dma_gather indices are int16 (max 32767) but nodes go to 49999, so h is
    viewed as [25000, 256] (two nodes per 1024B row) and each core's edges are
    bucketed by (src&1, dst&1) parity; the gather base AP selects the 512B
    half-row per bucket. Indices are node>>1 <= 24999.
  - Scores land as [128, cols] tiles; the host inverts the packing
    permutation and concatenates core outputs.

Self-contained: hardcodes N=50000, D=128, E=640000, 8 cores.
"""
import sys

if "/opt/trn_rl_repo" not in sys.path:
    sys.path.insert(0, "/opt/trn_rl_repo")

import numpy as np

N_NODES = 50000
D = 128
E = 640000
N_CORES = 8
P = 128
E_CORE = E // N_CORES            # 80000
CHUNK = 1024                     # edges per gather instruction (>=2048 crashes SWDGE)
N_BUCKETS = 4
CAP = 20992                      # per-bucket padded capacity (164*128); mean 20000, ~8 sigma margin
TOTAL = CAP * N_BUCKETS          # 83968 padded edge slots per core
TOTAL_COLS = TOTAL // P          # 656
IDX_COLS = TOTAL // 16           # 5248

_cache = {}


def _build_program(repeat=1):
    """Build + compile the 8-core SPMD Bass program. repeat>1 re-runs the
    whole per-core workload (for HW timing by delta)."""
    import concourse.bacc as bacc
    import concourse.tile as tile
    from concourse import mybir

    nc = bacc.Bacc("TRN2", target_bir_lowering=False, debug=False,
                   num_devices=N_CORES)
    h2_t = nc.dram_tensor("h2", [N_NODES // 2, 2 * D], mybir.dt.float32,
                          kind="ExternalInput")
    isrc_t = nc.dram_tensor("isrc", [P, IDX_COLS], mybir.dt.int16,
                            kind="ExternalInput")
    idst_t = nc.dram_tensor("idst", [P, IDX_COLS], mybir.dt.int16,
                            kind="ExternalInput")
    out_t = nc.dram_tensor("scores", [P, TOTAL_COLS], mybir.dt.float32,
                           kind="ExternalOutput")

    S = CHUNK // P

    with tile.TileContext(nc) as tc:
        with (
            tc.tile_pool(name="idx", bufs=1) as idx_pool,
            tc.tile_pool(name="gath", bufs=3) as gath_pool,
            tc.tile_pool(name="acc", bufs=1) as acc_pool,
        ):
            isrc = idx_pool.tile([P, IDX_COLS], mybir.dt.int16)
            idst = idx_pool.tile([P, IDX_COLS], mybir.dt.int16)
            nc.sync.dma_start(isrc[:], isrc_t[:])
            nc.sync.dma_start(idst[:], idst_t[:])
            scores = acc_pool.tile([P, TOTAL_COLS], mybir.dt.float32)

            for _ in range(repeat):
                for b in range(N_BUCKETS):
                    ps, pd = b >> 1, b & 1
                    for pos in range(0, CAP, CHUNK):
                        n = min(CHUNK, CAP - pos)
                        s_n = n // P
                        pos0 = b * CAP + pos
                        col0 = pos0 // P
                        icol0 = pos0 // 16
                        hu = gath_pool.tile([P, S, D], mybir.dt.float32, tag="hu")
                        hv = gath_pool.tile([P, S, D], mybir.dt.float32, tag="hv")
                        nc.gpsimd.dma_gather(
                            hu[:, :s_n], h2_t[:, ps * D:(ps + 1) * D],
                            isrc[:, icol0:icol0 + n // 16],
                            n, n, D, elem_step=2 * D, queue_num=0,
                        )
                        nc.gpsimd.dma_gather(
                            hv[:, :s_n], h2_t[:, pd * D:(pd + 1) * D],
                            idst[:, icol0:icol0 + n // 16],
                            n, n, D, elem_step=2 * D, queue_num=0,
                        )
                        prod = gath_pool.tile([P, S, D], mybir.dt.float32, tag="prod")
                        nc.vector.tensor_tensor(out=prod[:, :s_n], in0=hu[:, :s_n],
                                                in1=hv[:, :s_n],
                                                op=mybir.AluOpType.mult)
                        nc.vector.tensor_reduce(
                            out=scores[:, col0:col0 + s_n], in_=prod[:, :s_n],
                            axis=mybir.AxisListType.X, op=mybir.AluOpType.add,
                        )
            nc.sync.dma_start(out_t[:], scores[:])

    nc.compile()
    return nc


def get_program(repeat=1):
    if repeat not in _cache:
        _cache[repeat] = _build_program(repeat)
    return _cache[repeat]


def _idx_tile_layout(idx_flat):
    # [n] int16 -> [128, n//16]: position i at [i%16, i//16], replicated 8x
    n = len(idx_flat)
    t = idx_flat.reshape(n // 16, 16).T
    return np.tile(t, (8, 1))


# fixed unscramble map: padded position i -> (partition, column) of scores tile
_pos = np.arange(TOTAL)
_within = _pos % CAP
_chunk0 = (_within // CHUNK) * CHUNK
_r = _within - _chunk0
_POS_P = _r % P
_POS_COL = (_pos // CAP) * (CAP // P) + _chunk0 // P + _r // P


def _pack_core(src_c, dst_c):
    """Bucket one core's edges by parity pair; returns (isrc, idst, edge_order,
    spill_edges). Padded positions hold node 0 gathers (discarded)."""
    bucket = ((src_c & 1) * 2 + (dst_c & 1)).astype(np.int64)
    isrc_cols, idst_cols, order = [], [], []
    spill = []
    for b in range(N_BUCKETS):
        eb = np.nonzero(bucket == b)[0]
        if len(eb) > CAP:
            spill.append(eb[CAP:])
            eb = eb[:CAP]
        pad = CAP - len(eb)
        order.append(np.concatenate([eb, np.full(pad, -1, np.int64)]))
        s = np.concatenate([src_c[eb] >> 1, np.zeros(pad, np.int64)]).astype(np.int16)
        d = np.concatenate([dst_c[eb] >> 1, np.zeros(pad, np.int64)]).astype(np.int16)
        isrc_cols.append(_idx_tile_layout(s))
        idst_cols.append(_idx_tile_layout(d))
    return (
        np.concatenate(isrc_cols, axis=1),
        np.concatenate(idst_cols, axis=1),
        np.concatenate(order),
        np.concatenate(spill) if spill else None,
    )


def kernel(h, src, dst):
    from concourse.bass_utils import run_bass_kernel_spmd

    h = np.ascontiguousarray(np.asarray(h, dtype=np.float32))
    src = np.asarray(src).astype(np.int64)
    dst = np.asarray(dst).astype(np.int64)
    assert h.shape == (N_NODES, D) and src.shape == (E,) and dst.shape == (E,)

    nc = get_program(1)
    h2 = h.reshape(N_NODES // 2, 2 * D)

    in_maps, packs = [], []
    for c in range(N_CORES):
        sl = slice(c * E_CORE, (c + 1) * E_CORE)
        isrc_np, idst_np, order, spill = _pack_core(src[sl], dst[sl])
        packs.append((order, spill, sl))
        in_maps.append({"h2": h2, "isrc": isrc_np, "idst": idst_np})

    res = run_bass_kernel_spmd(nc, in_maps, list(range(N_CORES)))

    out = np.empty(E, dtype=np.float32)
    for c, (order, spill, sl) in enumerate(packs):
        scores_mat = res.results[c]["scores"]
        stream = scores_mat[_POS_P, _POS_COL]
        valid = order >= 0
        out[sl.start + order[valid]] = stream[valid]
        if spill is not None:  # statistically ~never: bucket overflow fallback
            gl = sl.start + spill
            out[gl] = (h[src[gl]] * h[dst[gl]]).sum(-1)
    return out


# revision 9
# speedup vs baseline: 19.0691x; 19.0691x over previous
"""Trainium2 Bass kernel for DotPredictor: score[e] = dot(h[src[e]], h[dst[e]]).

Strategy (8 NeuronCores, edge-parallel):
  - Shard the 640k edges across 8 cores (80k each).
  - Per core, gather h rows for src and dst with the SWDGE dma_gather
    instruction (vectorized Q7 descriptor generation, 512B rows from HBM),
    multiply elementwise on DVE and segmented-reduce over the feature axis.
  - dma_gather indices are int16 (max 32767) but nodes go to 49999, so h is
    viewed as [25000, 256] (two nodes per 1024B row) and each core's edges are
    bucketed by (src&1, dst&1) parity; the gather base AP selects the 512B
    half-row per bucket. Indices are node>>1 <= 24999.
  - Scores land as [128, cols] tiles; the host inverts the packing
    permutation and concatenates core outputs.

Self-contained: hardcodes N=50000, D=128, E=640000, 8 cores.
"""
import sys

if "/opt/trn_rl_repo" not in sys.path:
    sys.path.insert(0, "/opt/trn_rl_repo")

import numpy as np

N_NODES = 50000
D = 128
E = 640000
N_CORES = 8
P = 128
E_CORE = E // N_CORES            # 80000
CHUNK = 1024                     # edges per gather instruction (>=2048 crashes SWDGE)
N_BUCKETS = 4
CAP = 20992                      # per-bucket padded capacity (164*128); mean 20000, ~8 sigma margin
TOTAL = CAP * N_BUCKETS          # 83968 padded edge slots per core
TOTAL_COLS = TOTAL // P          # 656
IDX_COLS = TOTAL // 16           # 5248

_cache = {}


def _build_program(repeat=1, compute=True, bufs=3):
    """Build + compile the 8-core SPMD Bass program. repeat>1 re-runs the
    whole per-core workload (for HW timing by delta)."""
    import concourse.bacc as bacc
    import concourse.tile as tile
    from concourse import mybir

    nc = bacc.Bacc("TRN2", target_bir_lowering=False, debug=False,
                   num_devices=N_CORES)
    h2_t = nc.dram_tensor("h2", [N_NODES // 2, 2 * D], mybir.dt.float32,
                          kind="ExternalInput")
    isrc_t = nc.dram_tensor("isrc", [P, IDX_COLS], mybir.dt.int16,
                            kind="ExternalInput")
    idst_t = nc.dram_tensor("idst", [P, IDX_COLS], mybir.dt.int16,
                            kind="ExternalInput")
    out_t = nc.dram_tensor("scores", [P, TOTAL_COLS], mybir.dt.float32,
                           kind="ExternalOutput")

    S = CHUNK // P

    with tile.TileContext(nc) as tc:
        with (
            tc.tile_pool(name="idx", bufs=1) as idx_pool,
            tc.tile_pool(name="gath", bufs=bufs) as gath_pool,
            tc.tile_pool(name="acc", bufs=1) as acc_pool,
        ):
            isrc = idx_pool.tile([P, IDX_COLS], mybir.dt.int16)
            idst = idx_pool.tile([P, IDX_COLS], mybir.dt.int16)
            nc.sync.dma_start(isrc[:], isrc_t[:])
            nc.sync.dma_start(idst[:], idst_t[:])
            scores = acc_pool.tile([P, TOTAL_COLS], mybir.dt.float32)
            if not compute:
                nc.gpsimd.memset(scores[:], 0)

            import contextlib
            loop_cm = tc.For_i(0, repeat, 1) if repeat > 1 else contextlib.nullcontext()
            with loop_cm:
                for b in range(N_BUCKETS):
                    ps, pd = b >> 1, b & 1
                    for pos in range(0, CAP, CHUNK):
                        n = min(CHUNK, CAP - pos)
                        s_n = n // P
                        pos0 = b * CAP + pos
                        col0 = pos0 // P
                        icol0 = pos0 // 16
                        hu = gath_pool.tile([P, S, D], mybir.dt.float32, tag="hu")
                        hv = gath_pool.tile([P, S, D], mybir.dt.float32, tag="hv")
                        nc.gpsimd.dma_gather(
                            hu[:, :s_n], h2_t[:, ps * D:(ps + 1) * D],
                            isrc[:, icol0:icol0 + n // 16],
                            n, n, D, elem_step=2 * D, queue_num=0,
                        )
                        nc.gpsimd.dma_gather(
                            hv[:, :s_n], h2_t[:, pd * D:(pd + 1) * D],
                            idst[:, icol0:icol0 + n // 16],
                            n, n, D, elem_step=2 * D, queue_num=0,
                        )
                        if compute:
                            prod = gath_pool.tile([P, S, D], mybir.dt.float32, tag="prod")
                            nc.vector.tensor_tensor(out=prod[:, :s_n], in0=hu[:, :s_n],
                                                    in1=hv[:, :s_n],
                                                    op=mybir.AluOpType.mult)
                            nc.vector.tensor_reduce(
                                out=scores[:, col0:col0 + s_n], in_=prod[:, :s_n],
                                axis=mybir.AxisListType.X, op=mybir.AluOpType.add,
                            )
            nc.sync.dma_start(out_t[:], scores[:])

    nc.compile()
    return nc


def get_program(repeat=1, compute=True, bufs=3):
    key = (repeat, compute, bufs)
    if key not in _cache:
        _cache[key] = _build_program(repeat, compute, bufs)
    return _cache[key]


def _idx_tile_layout(idx_flat):
    # [n] int16 -> [128, n//16]: position i at [i%16, i//16], replicated 8x
    n = len(idx_flat)
    t = idx_flat.reshape(n // 16, 16).T
    return np.tile(t, (8, 1))


# fixed unscramble map: padded position i -> (partition, column) of scores tile
_pos = np.arange(TOTAL)
_within = _pos % CAP
_chunk0 = (_within // CHUNK) * CHUNK
_r = _within - _chunk0
_POS_P = _r % P
_POS_COL = (_pos // CAP) * (CAP // P) + _chunk0 // P + _r // P


def _pack_core(src_c, dst_c):
    """Bucket one core's edges by parity pair; returns (isrc, idst, edge_order,
    spill_edges). Padded positions hold node 0 gathers (discarded)."""
    bucket = ((src_c & 1) * 2 + (dst_c & 1)).astype(np.int64)
    isrc_cols, idst_cols, order = [], [], []
    spill = []
    for b in range(N_BUCKETS):
        eb = np.nonzero(bucket == b)[0]
        if len(eb) > CAP:
            spill.append(eb[CAP:])
            eb = eb[:CAP]
        pad = CAP - len(eb)
        order.append(np.concatenate([eb, np.full(pad, -1, np.int64)]))
        s = np.concatenate([src_c[eb] >> 1, np.zeros(pad, np.int64)]).astype(np.int16)
        d = np.concatenate([dst_c[eb] >> 1, np.zeros(pad, np.int64)]).astype(np.int16)
        isrc_cols.append(_idx_tile_layout(s))
        idst_cols.append(_idx_tile_layout(d))
    return (
        np.concatenate(isrc_cols, axis=1),
        np.concatenate(idst_cols, axis=1),
        np.concatenate(order),
        np.concatenate(spill) if spill else None,
    )


def kernel(h, src, dst):
    from concourse.bass_utils import run_bass_kernel_spmd

    h = np.ascontiguousarray(np.asarray(h, dtype=np.float32))
    src = np.asarray(src).astype(np.int64)
    dst = np.asarray(dst).astype(np.int64)
    assert h.shape == (N_NODES, D) and src.shape == (E,) and dst.shape == (E,)

    nc = get_program(1)
    h2 = h.reshape(N_NODES // 2, 2 * D)

    in_maps, packs = [], []
    for c in range(N_CORES):
        sl = slice(c * E_CORE, (c + 1) * E_CORE)
        isrc_np, idst_np, order, spill = _pack_core(src[sl], dst[sl])
        packs.append((order, spill, sl))
        in_maps.append({"h2": h2, "isrc": isrc_np, "idst": idst_np})

    res = run_bass_kernel_spmd(nc, in_maps, list(range(N_CORES)))

    out = np.empty(E, dtype=np.float32)
    for c, (order, spill, sl) in enumerate(packs):
        scores_mat = res.results[c]["scores"]
        stream = scores_mat[_POS_P, _POS_COL]
        valid = order >= 0
        out[sl.start + order[valid]] = stream[valid]
        if spill is not None:  # statistically ~never: bucket overflow fallback
            gl = sl.start + spill
            out[gl] = (h[src[gl]] * h[dst[gl]]).sum(-1)
    return out


# revision 10
# speedup vs baseline: 20.8914x; 1.0956x over previous
"""v2: dst-side pair sharing. Edges with IDENTICAL dst are paired; each pair's
hv row (512B) is gathered once and multiplied against both members' hu rows.

Regions per core (80k edges):
  P region: 8 pair-buckets keyed by (src_a&1, src_b&1, dst&1). Per chunk of
    512 pairs: hv gather (512 idxs, elem 512B, base=dst-parity) + huA gather
    (a-members) + huB gather. pair i -> slot (i%128, i//128) in all three
    tiles; pr_a = huA*hv, pr_b = huB*hv.
  U region: leftover unpaired edges, 4 parity buckets exactly like v1.

Scores: fixed (position -> partition, col) map mirrors emission order.
"""
import sys

if "/opt/trn_rl_repo" not in sys.path:
    sys.path.insert(0, "/opt/trn_rl_repo")

import numpy as np

N_NODES = 50000
D = 128
E = 640000
N_CORES = 8
P = 128
E_CORE = E // N_CORES            # 80000

PAIR_CAP = 3840                  # pairs per bucket; mean ~3501, +6 sigma
PAIR_CHUNK = 512
N_PBUCKETS = 8
U_CAP = 6400                     # unpaired edges per bucket; mean ~6000
U_CHUNK = 1024
N_UBUCKETS = 4

PAIR_SLOTS = PAIR_CAP * N_PBUCKETS * 2        # 61440 edge slots
U_SLOTS = U_CAP * N_UBUCKETS                  # 25600
TOTAL_COLS = (PAIR_SLOTS + U_SLOTS) // P      # 680

PB_ICOLS = 3 * (PAIR_CAP // 16)               # hv, huA, huB
UB_ICOLS = 2 * (U_CAP // 16)                  # hv, hu
IDX_COLS = N_PBUCKETS * PB_ICOLS + N_UBUCKETS * UB_ICOLS

_cache = {}


def _build_program(repeat=1):
    import concourse.bacc as bacc
    import concourse.tile as tile
    from concourse import mybir
    import contextlib

    nc = bacc.Bacc("TRN2", target_bir_lowering=False, debug=False,
                   num_devices=N_CORES)
    h2_t = nc.dram_tensor("h2", [N_NODES // 2, 2 * D], mybir.dt.float32,
                          kind="ExternalInput")
    idx_t = nc.dram_tensor("idx", [P, IDX_COLS], mybir.dt.int16,
                           kind="ExternalInput")
    out_t = nc.dram_tensor("scores", [P, TOTAL_COLS], mybir.dt.float32,
                           kind="ExternalOutput")

    with tile.TileContext(nc) as tc:
        with (
            tc.tile_pool(name="idx", bufs=1) as idx_pool,
            tc.tile_pool(name="gath", bufs=3) as gath_pool,
            tc.tile_pool(name="acc", bufs=1) as acc_pool,
        ):
            idx = idx_pool.tile([P, IDX_COLS], mybir.dt.int16)
            nc.sync.dma_start(idx[:], idx_t[:])
            scores = acc_pool.tile([P, TOTAL_COLS], mybir.dt.float32)

            loop_cm = tc.For_i(0, repeat, 1) if repeat > 1 else contextlib.nullcontext()
            with loop_cm:
                col = 0
                # ---- P region ----
                for b in range(N_PBUCKETS):
                    sa, sb_, d = (b >> 2) & 1, (b >> 1) & 1, b & 1
                    ic0 = b * PB_ICOLS
                    ic_hv = ic0
                    ic_a = ic0 + PAIR_CAP // 16
                    ic_b = ic0 + 2 * (PAIR_CAP // 16)
                    for pos in range(0, PAIR_CAP, PAIR_CHUNK):
                        n = min(PAIR_CHUNK, PAIR_CAP - pos)
                        s_n = n // P
                        o = pos // 16
                        hv = gath_pool.tile([P, PAIR_CHUNK // P, D],
                                            mybir.dt.float32, tag="hvp")
                        hua = gath_pool.tile([P, PAIR_CHUNK // P, D],
                                             mybir.dt.float32, tag="hua")
                        hub = gath_pool.tile([P, PAIR_CHUNK // P, D],
                                             mybir.dt.float32, tag="hub")
                        nc.gpsimd.dma_gather(
                            hv[:, :s_n], h2_t[:, d * D:(d + 1) * D],
                            idx[:, ic_hv + o:ic_hv + o + n // 16],
                            n, n, D, elem_step=2 * D, queue_num=0,
                        )
                        nc.gpsimd.dma_gather(
                            hua[:, :s_n], h2_t[:, sa * D:(sa + 1) * D],
                            idx[:, ic_a + o:ic_a + o + n // 16],
                            n, n, D, elem_step=2 * D, queue_num=0,
                        )
                        nc.gpsimd.dma_gather(
                            hub[:, :s_n], h2_t[:, sb_ * D:(sb_ + 1) * D],
                            idx[:, ic_b + o:ic_b + o + n // 16],
                            n, n, D, elem_step=2 * D, queue_num=0,
                        )
                        pr_a = gath_pool.tile([P, PAIR_CHUNK // P, D],
                                              mybir.dt.float32, tag="pra")
                        pr_b = gath_pool.tile([P, PAIR_CHUNK // P, D],
                                              mybir.dt.float32, tag="prb")
                        nc.vector.tensor_tensor(out=pr_a[:, :s_n], in0=hua[:, :s_n],
                                                in1=hv[:, :s_n],
                                                op=mybir.AluOpType.mult)
                        nc.vector.tensor_tensor(out=pr_b[:, :s_n], in0=hub[:, :s_n],
                                                in1=hv[:, :s_n],
                                                op=mybir.AluOpType.mult)
                        nc.vector.tensor_reduce(
                            out=scores[:, col:col + s_n], in_=pr_a[:, :s_n],
                            axis=mybir.AxisListType.X, op=mybir.AluOpType.add)
                        nc.vector.tensor_reduce(
                            out=scores[:, col + s_n:col + 2 * s_n], in_=pr_b[:, :s_n],
                            axis=mybir.AxisListType.X, op=mybir.AluOpType.add)
                        col += 2 * s_n
                # ---- U region ----
                u_base = N_PBUCKETS * PB_ICOLS
                for b in range(N_UBUCKETS):
                    ps, pd = b >> 1, b & 1
                    ic0 = u_base + b * UB_ICOLS
                    ic_hv = ic0
                    ic_hu = ic0 + U_CAP // 16
                    for pos in range(0, U_CAP, U_CHUNK):
                        n = min(U_CHUNK, U_CAP - pos)
                        s_n = n // P
                        o = pos // 16
                        hu = gath_pool.tile([P, U_CHUNK // P, D],
                                            mybir.dt.float32, tag="huu")
                        hvu = gath_pool.tile([P, U_CHUNK // P, D],
                                             mybir.dt.float32, tag="hvu")
                        nc.gpsimd.dma_gather(
                            hu[:, :s_n], h2_t[:, ps * D:(ps + 1) * D],
                            idx[:, ic_hu + o:ic_hu + o + n // 16],
                            n, n, D, elem_step=2 * D, queue_num=0,
                        )
                        nc.gpsimd.dma_gather(
                            hvu[:, :s_n], h2_t[:, pd * D:(pd + 1) * D],
                            idx[:, ic_hv + o:ic_hv + o + n // 16],
                            n, n, D, elem_step=2 * D, queue_num=0,
                        )
                        pr = gath_pool.tile([P, U_CHUNK // P, D],
                                            mybir.dt.float32, tag="pru")
                        nc.vector.tensor_tensor(out=pr[:, :s_n], in0=hu[:, :s_n],
                                                in1=hvu[:, :s_n],
                                                op=mybir.AluOpType.mult)
                        nc.vector.tensor_reduce(
                            out=scores[:, col:col + s_n], in_=pr[:, :s_n],
                            axis=mybir.AxisListType.X, op=mybir.AluOpType.add)
                        col += s_n
                assert col == TOTAL_COLS, col
            nc.sync.dma_start(out_t[:], scores[:])

    nc.compile()
    return nc


def get_program(repeat=1):
    if repeat not in _cache:
        _cache[repeat] = _build_program(repeat)
    return _cache[repeat]


def _tile16(idx_flat):
    n = len(idx_flat)
    t = np.asarray(idx_flat, np.int16).reshape(n // 16, 16).T
    return np.tile(t, (8, 1))


def _pack_core(src_c, dst_c):
    E_c = len(src_c)
    # group by identical dst: full pairs -> P region, leftovers -> U
    order_r = np.argsort(dst_c, kind="stable")
    ds = dst_c[order_r]
    is_new = np.empty(E_c, bool)
    is_new[0] = True
    is_new[1:] = ds[1:] != ds[:-1]
    run_id = np.cumsum(is_new) - 1
    run_start = np.maximum.accumulate(np.where(is_new, np.arange(E_c), 0))
    pos_in_run = np.arange(E_c) - run_start
    run_len = np.bincount(run_id)
    my_len = run_len[run_id]
    paired_mask = pos_in_run < (my_len - (my_len & 1))
    paired = order_r[paired_mask]          # consecutive entries share dst
    unpaired = order_r[~paired_mask]
    pa_e, pb_e = paired[0::2], paired[1::2]

    pkey = (((src_c[pa_e] & 1) << 2) | ((src_c[pb_e] & 1) << 1)
            | (dst_c[pa_e] & 1)).astype(np.int64)
    ukey = ((src_c[unpaired] & 1) * 2 + (dst_c[unpaired] & 1)).astype(np.int64)

    idx_cols = []
    edge_order = np.full(PAIR_SLOTS + U_SLOTS, -1, np.int64)
    spill = []

    for b in range(N_PBUCKETS):
        sel = np.nonzero(pkey == b)[0]
        if len(sel) > PAIR_CAP:
            for e in sel[PAIR_CAP:]:
                spill.append(pa_e[e]); spill.append(pb_e[e])
            sel = sel[:PAIR_CAP]
        pad = PAIR_CAP - len(sel)
        a_edges, b_edges = pa_e[sel], pb_e[sel]
        z = np.zeros(pad, np.int64)
        idx_cols += [
            _tile16(np.concatenate([dst_c[a_edges] >> 1, z])),
            _tile16(np.concatenate([src_c[a_edges] >> 1, z])),
            _tile16(np.concatenate([src_c[b_edges] >> 1, z])),
        ]
        base = b * PAIR_CAP * 2
        for pos in range(0, PAIR_CAP, PAIR_CHUNK):
            n = min(PAIR_CHUNK, PAIR_CAP - pos)
            blk = base + 2 * pos
            k_hi = min(len(a_edges), pos + n)
            if k_hi > pos:
                m = k_hi - pos
                edge_order[blk:blk + m] = a_edges[pos:k_hi]
                edge_order[blk + n:blk + n + m] = b_edges[pos:k_hi]
    u_pos_base = PAIR_SLOTS
    for b in range(N_UBUCKETS):
        sel = np.nonzero(ukey == b)[0]
        if len(sel) > U_CAP:
            spill.extend(unpaired[sel[U_CAP:]])
            sel = sel[:U_CAP]
        pad = U_CAP - len(sel)
        ue = unpaired[sel]
        z = np.zeros(pad, np.int64)
        idx_cols += [
            _tile16(np.concatenate([dst_c[ue] >> 1, z])),
            _tile16(np.concatenate([src_c[ue] >> 1, z])),
        ]
        base = u_pos_base + b * U_CAP
        edge_order[base:base + len(ue)] = ue

    idx_np = np.concatenate(idx_cols, axis=1)
    assert idx_np.shape == (P, IDX_COLS), idx_np.shape
    return idx_np, edge_order, (np.array(spill, np.int64) if spill else None)


def _build_posmap():
    pmap = np.empty(PAIR_SLOTS + U_SLOTS, np.int64)
    cmap = np.empty(PAIR_SLOTS + U_SLOTS, np.int64)
    col = 0
    pos = 0
    for b in range(N_PBUCKETS):
        for cpos in range(0, PAIR_CAP, PAIR_CHUNK):
            n = min(PAIR_CHUNK, PAIR_CAP - cpos)
            s_n = n // P
            i = np.arange(n)
            pmap[pos:pos + n] = i % P
            cmap[pos:pos + n] = col + i // P
            pos += n
            pmap[pos:pos + n] = i % P
            cmap[pos:pos + n] = col + s_n + i // P
            pos += n
            col += 2 * s_n
    for b in range(N_UBUCKETS):
        for cpos in range(0, U_CAP, U_CHUNK):
            n = min(U_CHUNK, U_CAP - cpos)
            s_n = n // P
            i = np.arange(n)
            pmap[pos:pos + n] = i % P
            cmap[pos:pos + n] = col + i // P
            pos += n
            col += s_n
    assert col == TOTAL_COLS and pos == PAIR_SLOTS + U_SLOTS
    return pmap, cmap


_POS_P, _POS_COL = _build_posmap()


def kernel(h, src, dst):
    from concourse.bass_utils import run_bass_kernel_spmd

    h = np.ascontiguousarray(np.asarray(h, dtype=np.float32))
    src = np.asarray(src).astype(np.int64)
    dst = np.asarray(dst).astype(np.int64)

    nc = get_program(1)
    h2 = h.reshape(N_NODES // 2, 2 * D)
    in_maps, packs = [], []
    for c in range(N_CORES):
        sl = slice(c * E_CORE, (c + 1) * E_CORE)
        idx_np, order, spill = _pack_core(src[sl], dst[sl])
        packs.append((order, spill, sl))
        in_maps.append({"h2": h2, "idx": idx_np})

    res = run_bass_kernel_spmd(nc, in_maps, list(range(N_CORES)))

    out = np.empty(E, dtype=np.float32)
    for c, (order, spill, sl) in enumerate(packs):
        scores_mat = res.results[c]["scores"]
        stream = scores_mat[_POS_P, _POS_COL]
        valid = order >= 0
        out[sl.start + order[valid]] = stream[valid]
        if spill is not None:
            gl = sl.start + spill
            out[gl] = (h[src[gl]] * h[dst[gl]]).sum(-1)
    return out


# revision 12
# speedup vs baseline: 21.2841x; 1.0188x over previous
"""v2: dst-side pair sharing. Edges with IDENTICAL dst are paired; each pair's
hv row (512B) is gathered once and multiplied against both members' hu rows.

Regions per core (80k edges):
  P region: 8 pair-buckets keyed by (src_a&1, src_b&1, dst&1). Per chunk of
    512 pairs: hv gather (512 idxs, elem 512B, base=dst-parity) + huA gather
    (a-members) + huB gather. pair i -> slot (i%128, i//128) in all three
    tiles; pr_a = huA*hv, pr_b = huB*hv.
  U region: leftover unpaired edges, 4 parity buckets exactly like v1.

Scores: fixed (position -> partition, col) map mirrors emission order.
"""
import sys

if "/opt/trn_rl_repo" not in sys.path:
    sys.path.insert(0, "/opt/trn_rl_repo")

import numpy as np

N_NODES = 50000
D = 128
E = 640000
N_CORES = 8
P = 128
E_CORE = E // N_CORES            # 80000

PAIR_CAP = 3840                  # pairs per bucket; mean ~3501, +6 sigma
PAIR_CHUNK = 512
N_PBUCKETS = 8
U_CAP = 6400                     # unpaired edges per bucket; mean ~6000
U_CHUNK = 1024
N_UBUCKETS = 4

PAIR_SLOTS = PAIR_CAP * N_PBUCKETS * 2        # 81920 edge slots
U_SLOTS = U_CAP * N_UBUCKETS                  # 4096
TOTAL_COLS = (PAIR_SLOTS + U_SLOTS) // P      # 672

PB_ICOLS = 3 * (PAIR_CAP // 16)               # hv, huA, huB
UB_ICOLS = 2 * (U_CAP // 16)                  # hv, hu
IDX_COLS = N_PBUCKETS * PB_ICOLS + N_UBUCKETS * UB_ICOLS

_cache = {}


def _build_program(repeat=1):
    import concourse.bacc as bacc
    import concourse.tile as tile
    from concourse import mybir
    import contextlib

    nc = bacc.Bacc("TRN2", target_bir_lowering=False, debug=False,
                   num_devices=N_CORES)
    h2_t = nc.dram_tensor("h2", [N_NODES // 2, 2 * D], mybir.dt.float32,
                          kind="ExternalInput")
    idx_t = nc.dram_tensor("idx", [P, IDX_COLS], mybir.dt.int16,
                           kind="ExternalInput")
    out_t = nc.dram_tensor("scores", [P, TOTAL_COLS], mybir.dt.float32,
                           kind="ExternalOutput")

    with tile.TileContext(nc) as tc:
        with (
            tc.tile_pool(name="idx", bufs=1) as idx_pool,
            tc.tile_pool(name="gath", bufs=3) as gath_pool,
            tc.tile_pool(name="acc", bufs=1) as acc_pool,
        ):
            idx = idx_pool.tile([P, IDX_COLS], mybir.dt.int16)
            nc.sync.dma_start(idx[:], idx_t[:])
            scores = acc_pool.tile([P, TOTAL_COLS], mybir.dt.float32)

            loop_cm = tc.For_i(0, repeat, 1) if repeat > 1 else contextlib.nullcontext()
            with loop_cm:
                col = 0
                # ---- P region ----
                for b in range(N_PBUCKETS):
                    sa, sb_, d = (b >> 2) & 1, (b >> 1) & 1, b & 1
                    ic0 = b * PB_ICOLS
                    ic_hv = ic0
                    ic_a = ic0 + PAIR_CAP // 16
                    ic_b = ic0 + 2 * (PAIR_CAP // 16)
                    for pos in range(0, PAIR_CAP, PAIR_CHUNK):
                        n = min(PAIR_CHUNK, PAIR_CAP - pos)
                        s_n = n // P
                        o = pos // 16
                        hv = gath_pool.tile([P, PAIR_CHUNK // P, D],
                                            mybir.dt.float32, tag="hvp")
                        hua = gath_pool.tile([P, PAIR_CHUNK // P, D],
                                             mybir.dt.float32, tag="hua")
                        hub = gath_pool.tile([P, PAIR_CHUNK // P, D],
                                             mybir.dt.float32, tag="hub")
                        nc.gpsimd.dma_gather(
                            hv[:, :s_n], h2_t[:, d * D:(d + 1) * D],
                            idx[:, ic_hv + o:ic_hv + o + n // 16],
                            n, n, D, elem_step=2 * D, queue_num=0,
                        )
                        nc.gpsimd.dma_gather(
                            hua[:, :s_n], h2_t[:, sa * D:(sa + 1) * D],
                            idx[:, ic_a + o:ic_a + o + n // 16],
                            n, n, D, elem_step=2 * D, queue_num=0,
                        )
                        nc.gpsimd.dma_gather(
                            hub[:, :s_n], h2_t[:, sb_ * D:(sb_ + 1) * D],
                            idx[:, ic_b + o:ic_b + o + n // 16],
                            n, n, D, elem_step=2 * D, queue_num=0,
                        )
                        pr_a = gath_pool.tile([P, PAIR_CHUNK // P, D],
                                              mybir.dt.float32, tag="pra")
                        pr_b = gath_pool.tile([P, PAIR_CHUNK // P, D],
                                              mybir.dt.float32, tag="prb")
                        nc.vector.tensor_tensor(out=pr_a[:, :s_n], in0=hua[:, :s_n],
                                                in1=hv[:, :s_n],
                                                op=mybir.AluOpType.mult)
                        nc.vector.tensor_tensor(out=pr_b[:, :s_n], in0=hub[:, :s_n],
                                                in1=hv[:, :s_n],
                                                op=mybir.AluOpType.mult)
                        nc.vector.tensor_reduce(
                            out=scores[:, col:col + s_n], in_=pr_a[:, :s_n],
                            axis=mybir.AxisListType.X, op=mybir.AluOpType.add)
                        nc.vector.tensor_reduce(
                            out=scores[:, col + s_n:col + 2 * s_n], in_=pr_b[:, :s_n],
                            axis=mybir.AxisListType.X, op=mybir.AluOpType.add)
                        col += 2 * s_n
                # ---- U region ----
                u_base = N_PBUCKETS * PB_ICOLS
                for b in range(N_UBUCKETS):
                    ps, pd = b >> 1, b & 1
                    ic0 = u_base + b * UB_ICOLS
                    ic_hv = ic0
                    ic_hu = ic0 + U_CAP // 16
                    for pos in range(0, U_CAP, U_CHUNK):
                        n = min(U_CHUNK, U_CAP - pos)
                        s_n = n // P
                        o = pos // 16
                        hu = gath_pool.tile([P, U_CHUNK // P, D],
                                            mybir.dt.float32, tag="huu")
                        hvu = gath_pool.tile([P, U_CHUNK // P, D],
                                             mybir.dt.float32, tag="hvu")
                        nc.gpsimd.dma_gather(
                            hu[:, :s_n], h2_t[:, ps * D:(ps + 1) * D],
                            idx[:, ic_hu + o:ic_hu + o + n // 16],
                            n, n, D, elem_step=2 * D, queue_num=0,
                        )
                        nc.gpsimd.dma_gather(
                            hvu[:, :s_n], h2_t[:, pd * D:(pd + 1) * D],
                            idx[:, ic_hv + o:ic_hv + o + n // 16],
                            n, n, D, elem_step=2 * D, queue_num=0,
                        )
                        pr = gath_pool.tile([P, U_CHUNK // P, D],
                                            mybir.dt.float32, tag="pru")
                        nc.vector.tensor_tensor(out=pr[:, :s_n], in0=hu[:, :s_n],
                                                in1=hvu[:, :s_n],
                                                op=mybir.AluOpType.mult)
                        nc.vector.tensor_reduce(
                            out=scores[:, col:col + s_n], in_=pr[:, :s_n],
                            axis=mybir.AxisListType.X, op=mybir.AluOpType.add)
                        col += s_n
                assert col == TOTAL_COLS, col
            nc.sync.dma_start(out_t[:], scores[:])

    nc.compile()
    return nc


def get_program(repeat=1):
    if repeat not in _cache:
        _cache[repeat] = _build_program(repeat)
    return _cache[repeat]


def _tile16(idx_flat):
    n = len(idx_flat)
    t = np.asarray(idx_flat, np.int16).reshape(n // 16, 16).T
    return np.tile(t, (8, 1))


def _pack_core(src_c, dst_c):
    E_c = len(src_c)
    # group by identical dst: full pairs -> P region, leftovers -> U
    order_r = np.argsort(dst_c, kind="stable")
    ds = dst_c[order_r]
    is_new = np.empty(E_c, bool)
    is_new[0] = True
    is_new[1:] = ds[1:] != ds[:-1]
    run_id = np.cumsum(is_new) - 1
    run_start = np.maximum.accumulate(np.where(is_new, np.arange(E_c), 0))
    pos_in_run = np.arange(E_c) - run_start
    run_len = np.bincount(run_id)
    my_len = run_len[run_id]
    paired_mask = pos_in_run < (my_len - (my_len & 1))
    paired = order_r[paired_mask]          # consecutive entries share dst
    unpaired = order_r[~paired_mask]
    pa_e, pb_e = paired[0::2], paired[1::2]

    pkey = (((src_c[pa_e] & 1) << 2) | ((src_c[pb_e] & 1) << 1)
            | (dst_c[pa_e] & 1)).astype(np.int64)
    ukey = ((src_c[unpaired] & 1) * 2 + (dst_c[unpaired] & 1)).astype(np.int64)

    idx_cols = []
    edge_order = np.full(PAIR_SLOTS + U_SLOTS, -1, np.int64)
    spill = []

    for b in range(N_PBUCKETS):
        sel = np.nonzero(pkey == b)[0]
        if len(sel) > PAIR_CAP:
            for e in sel[PAIR_CAP:]:
                spill.append(pa_e[e]); spill.append(pb_e[e])
            sel = sel[:PAIR_CAP]
        pad = PAIR_CAP - len(sel)
        a_edges, b_edges = pa_e[sel], pb_e[sel]
        z = np.zeros(pad, np.int64)
        idx_cols += [
            _tile16(np.concatenate([dst_c[a_edges] >> 1, z])),
            _tile16(np.concatenate([src_c[a_edges] >> 1, z])),
            _tile16(np.concatenate([src_c[b_edges] >> 1, z])),
        ]
        base = b * PAIR_CAP * 2
        for pos in range(0, PAIR_CAP, PAIR_CHUNK):
            n = min(PAIR_CHUNK, PAIR_CAP - pos)
            blk = base + 2 * pos
            k_hi = min(len(a_edges), pos + n)
            if k_hi > pos:
                m = k_hi - pos
                edge_order[blk:blk + m] = a_edges[pos:k_hi]
                edge_order[blk + n:blk + n + m] = b_edges[pos:k_hi]
    u_pos_base = PAIR_SLOTS
    for b in range(N_UBUCKETS):
        sel = np.nonzero(ukey == b)[0]
        if len(sel) > U_CAP:
            spill.extend(unpaired[sel[U_CAP:]])
            sel = sel[:U_CAP]
        pad = U_CAP - len(sel)
        ue = unpaired[sel]
        z = np.zeros(pad, np.int64)
        idx_cols += [
            _tile16(np.concatenate([dst_c[ue] >> 1, z])),
            _tile16(np.concatenate([src_c[ue] >> 1, z])),
        ]
        base = u_pos_base + b * U_CAP
        edge_order[base:base + len(ue)] = ue

    idx_np = np.concatenate(idx_cols, axis=1)
    assert idx_np.shape == (P, IDX_COLS), idx_np.shape
    return idx_np, edge_order, (np.array(spill, np.int64) if spill else None)


def _build_posmap():
    pmap = np.empty(PAIR_SLOTS + U_SLOTS, np.int64)
    cmap = np.empty(PAIR_SLOTS + U_SLOTS, np.int64)
    col = 0
    pos = 0
    for b in range(N_PBUCKETS):
        for cpos in range(0, PAIR_CAP, PAIR_CHUNK):
            n = min(PAIR_CHUNK, PAIR_CAP - cpos)
            s_n = n // P
            i = np.arange(n)
            pmap[pos:pos + n] = i % P
            cmap[pos:pos + n] = col + i // P
            pos += n
            pmap[pos:pos + n] = i % P
            cmap[pos:pos + n] = col + s_n + i // P
            pos += n
            col += 2 * s_n
    for b in range(N_UBUCKETS):
        for cpos in range(0, U_CAP, U_CHUNK):
            n = min(U_CHUNK, U_CAP - cpos)
            s_n = n // P
            i = np.arange(n)
            pmap[pos:pos + n] = i % P
            cmap[pos:pos + n] = col + i // P
            pos += n
            col += s_n
    assert col == TOTAL_COLS and pos == PAIR_SLOTS + U_SLOTS
    return pmap, cmap


_POS_P, _POS_COL = _build_posmap()


def _shard_edges(src, dst):
    """Shard edges by dst range (6250 nodes per core) so identical-dst pairing
    is dense. Returns per core (global_edge_ids, src_c, dst_c)."""
    ca = dst // (N_NODES // N_CORES)
    shards = []
    for c in range(N_CORES):
        gids = np.nonzero(ca == c)[0]
        shards.append((gids, src[gids], dst[gids]))
    return shards


def kernel(h, src, dst):
    from concourse.bass_utils import run_bass_kernel_spmd

    h = np.ascontiguousarray(np.asarray(h, dtype=np.float32))
    src = np.asarray(src).astype(np.int64)
    dst = np.asarray(dst).astype(np.int64)

    nc = get_program(1)
    h2 = h.reshape(N_NODES // 2, 2 * D)
    in_maps, packs = [], []
    for gids, src_c, dst_c in _shard_edges(src, dst):
        idx_np, order, spill = _pack_core(src_c, dst_c)
        packs.append((order, spill, gids))
        in_maps.append({"h2": h2, "idx": idx_np})

    res = run_bass_kernel_spmd(nc, in_maps, list(range(N_CORES)))

    out = np.empty(E, dtype=np.float32)
    for c, (order, spill, gids) in enumerate(packs):
        scores_mat = res.results[c]["scores"]
        stream = scores_mat[_POS_P, _POS_COL]
        valid = order >= 0
        out[gids[order[valid]]] = stream[valid]
        if spill is not None:
            gl = gids[spill]
            out[gl] = (h[src[gl]] * h[dst[gl]]).sum(-1)
    return out
